# revision 11
# baseline (speedup 1.0000x reference)
"""Trainium2 Bass kernel for the channel-interaction-attention module.

Reference computation (x: (4, 1024, 64, 64) fp32, F = x.ravel()):
    A  = F.view(16384, 1024)          # x.reshape(-1, C)
    Bm = F.view(1024, 16384)          # x.reshape(C, -1)
    S  = Bm @ A                       # (C, C)
    E  = softmax(S, axis=-1)
    U  = E @ Bm                       # (C, N)
    Y  = softmax(U, axis=-1)          # softmax over N = 16384
    out = x + softmax(Y.view(4,1024,64,64), axis=-1)   # softmax over W=64
"""

import numpy as np
import ml_dtypes

import concourse.bass as bass
import concourse.bacc as bacc
import concourse.tile as tile
import concourse.mybir as mybir
from concourse import bass_utils

N_CORES = 8
B, C, H, W = 4, 1024, 64, 64
N = B * H * W            # 16384
NS = N // N_CORES        # 2048 per-core shard
P = 128
MT = C // P              # 8 row-blocks of S / U
KT1 = NS // P            # 16 contraction tiles for GEMM1
KT2 = C // P             # 8 contraction tiles for GEMM2
D1 = KT1 // 2            # 8 DoubleRow steps, GEMM1
D2 = KT2 // 2            # 4 DoubleRow steps, GEMM2

FP32 = mybir.dt.float32
BF16 = mybir.dt.bfloat16
FP8 = mybir.dt.float8e4
EXP = mybir.ActivationFunctionType.Exp
DR = mybir.MatmulPerfMode.DoubleRow
AX = mybir.AxisListType.X


def build_module(repeat: int = 1, fp8: bool = True, collectives: bool = True):
    nc = bacc.Bacc("TRN2", target_bir_lowering=False, debug=False,
                   num_devices=N_CORES if collectives else 1)

    def all_reduce(cc_in, cc_out):
        if collectives:
            nc.gpsimd.collective_compute(
                "AllReduce", mybir.AluOpType.add,
                replica_groups=[list(range(N_CORES))],
                ins=[cc_in.opt()], outs=[cc_out.opt()],
            )
        else:
            nc.sync.dma_start(cc_out[:], cc_in[:])

    a_d = nc.dram_tensor("a_in", [NS, C], FP8, kind="ExternalInput")
    bt_d = nc.dram_tensor("bt_in", [NS, C], FP8, kind="ExternalInput")
    b_d = nc.dram_tensor("b_in", [C, NS], FP8, kind="ExternalInput")
    o_d = nc.dram_tensor("o_out", [C, NS], BF16, kind="ExternalOutput")

    with tile.TileContext(nc) as tc:
        with (
            tc.tile_pool(name="lp", bufs=2) as lp,
            tc.tile_pool(name="upool", bufs=2) as upool,
            tc.tile_pool(name="etbfp", bufs=1) as etbfp,
            tc.tile_pool(name="etp", bufs=1) as etp,
            tc.tile_pool(name="ep", bufs=3) as ep,
            tc.tile_pool(name="srp", bufs=3) as srp,
            tc.tile_pool(name="scp", bufs=4) as scp,
            tc.tile_pool(name="zp", bufs=2) as zp,
            tc.tile_pool(name="wst", bufs=4) as wst,
            tc.tile_pool(name="stat", bufs=2) as stat,
            tc.tile_pool(name="cst", bufs=1) as cst,
            tc.tile_pool(name="ps1", bufs=2, space="PSUM") as psp1,
            tc.tile_pool(name="ps2", bufs=2, space="PSUM") as psp2,
            tc.tile_pool(name="dram", bufs=1, space="DRAM") as dram,
        ):
            ubias = cst.tile([P, 1], FP32, tag="ubias")
            nc.vector.memset(ubias[:], -1.5)
            for rep in range(repeat):
                # ---- stream in this rep's operands (prev rep overlaps) ----
                a_t = lp.tile([P, KT1, C], FP8, tag="a")
                bt_t = lp.tile([P, KT1, C], FP8, tag="bt")
                b_t = lp.tile([P, KT2, NS], FP8, tag="b")
                nc.sync.dma_start(
                    a_t[:], a_d[:].rearrange("(k p) c -> p k c", p=P))
                nc.sync.dma_start(
                    bt_t[:], bt_d[:].rearrange("(k p) c -> p k c", p=P))
                nc.sync.dma_start(
                    b_t[:], b_d[:].rearrange("(k p) n -> p k n", p=P))

                # ---- GEMM1: partial S/8 row-blocks, AllReduce by halves ----
                s_in = [dram.tile([P, 4, C], FP8, tag=f"ci{rep}_{h}",
                                  name=f"s_in{rep}_{h}") for h in range(2)]
                s_out = [dram.tile([P, 4, C], FP8, tag=f"co{rep}_{h}",
                                   addr_space="Shared",
                                   name=f"s_out{rep}_{h}") for h in range(2)]
                for m in range(MT):
                    ps = psp1.tile([P, C], FP32, tag="ps1",
                                   name=f"ps1_{rep}_{m}")
                    for k in range(D1):
                        for nn in range(2):
                            nc.tensor.matmul(
                                ps[:, nn * 512:(nn + 1) * 512],
                                bt_t[:, 2 * k:2 * k + 2,
                                     m * P:(m + 1) * P],
                                a_t[:, 2 * k:2 * k + 2,
                                    nn * 512:(nn + 1) * 512],
                                start=(k == 0), stop=(k == D1 - 1),
                                perf_mode=DR)
                    sc = scp.tile([P, C], FP8, tag="sc",
                                  name=f"sc_{rep}_{m}")
                    nc.vector.tensor_copy(sc[:], ps[:])
                    nc.sync.dma_start(s_in[m // 4][:, m % 4, :], sc[:])
                    if m % 4 == 3:
                        all_reduce(s_in[m // 4], s_out[m // 4])

                # ---- per row-block: softmax(S), E^T via DMA-transpose,
                #      GEMM2, exp-evict U ----
                negmax = stat.tile([P, MT], FP32, tag="nm")
                negmax8 = stat.tile([P, MT], FP32, tag="nm8")
                rsum = stat.tile([P, MT], FP32, tag="rs")
                rscale = stat.tile([P, MT], FP32, tag="rsc")
                acc = stat.tile([P, MT, 2], FP32, tag="ac")
                lsum = stat.tile([P, MT], FP32, tag="ls")
                gsum = stat.tile([P, MT], FP32, tag="gs")
                gscale = stat.tile([P, MT], FP32, tag="gsc")
                et_bf = etbfp.tile([P, KT2, C], BF16, tag="etbf")
                et_t = etp.tile([P, KT2, C], FP8, tag="et")
                u_t = upool.tile([P, MT, NS], FP8, tag="u")
                for m in range(MT):
                    sr = srp.tile([P, C], FP8, tag="sr", name=f"sr_{rep}_{m}")
                    nc.sync.dma_start(sr[:], s_out[m // 4][:, m % 4, :])
                    nc.vector.tensor_reduce(
                        negmax[:, m:m + 1], sr[:], axis=AX,
                        op=mybir.AluOpType.max, negate=True)
                    nc.vector.tensor_scalar_mul(
                        negmax8[:, m:m + 1], negmax[:, m:m + 1], 8.0)
                    e_t = ep.tile([P, C], BF16, tag="e", name=f"e_{rep}_{m}")
                    nc.scalar.activation(
                        e_t[:], sr[:], EXP,
                        bias=negmax8[:, m:m + 1], scale=8.0,
                        accum_out=rsum[:, m:m + 1])
                    nc.vector.reciprocal(rscale[:, m:m + 1], rsum[:, m:m + 1])
                    nc.sync.dma_start(et_bf[:, :, m * P:(m + 1) * P], e_t[:],
                                      transpose=True)
                    nc.vector.tensor_copy(et_t[:, :, m * P:(m + 1) * P],
                                          et_bf[:, :, m * P:(m + 1) * P])
                    # u = exp(rscale*Uraw - 1.5); the -1.5 keeps exp within
                    # fp8e4 range and cancels in the softmax normalization
                    for np_ in range(2):
                        ps2 = psp2.tile([P, C], FP32, tag="ps2",
                                        name=f"ps2_{rep}_{m}_{np_}")
                        for k in range(D2):
                            for nn in range(2):
                                nc.tensor.matmul(
                                    ps2[:, nn * 512:(nn + 1) * 512],
                                    et_t[:, 2 * k:2 * k + 2,
                                         m * P:(m + 1) * P],
                                    b_t[:, 2 * k:2 * k + 2,
                                        np_ * C + nn * 512:
                                        np_ * C + (nn + 1) * 512],
                                    start=(k == 0), stop=(k == D2 - 1),
                                    perf_mode=DR)
                        nc.scalar.activation(
                            u_t[:, m, np_ * C:(np_ + 1) * C], ps2[:], EXP,
                            bias=ubias[:], scale=rscale[:, m:m + 1],
                            accum_out=acc[:, m, np_:np_ + 1])

                # ---- N-softmax denominators: one tiny AllReduce ----
                ls_in = dram.tile([P, MT], FP32, tag=f"li{rep}",
                                  name=f"ls_in{rep}")
                ls_out = dram.tile([P, MT], FP32, tag=f"lo{rep}",
                                   addr_space="Shared", name=f"ls_out{rep}")
                nc.vector.tensor_reduce(lsum[:], acc[:], axis=AX,
                                        op=mybir.AluOpType.add)
                nc.sync.dma_start(ls_in[:], lsum[:])
                all_reduce(ls_in, ls_out)
                nc.sync.dma_start(gsum[:], ls_out[:])
                nc.vector.reciprocal(gscale[:], gsum[:])

                # ---- z = softmax_W(u * gscale) ; store ----
                for m in range(MT):
                    z = zp.tile([P, NS], BF16, tag="z", name=f"z_{rep}_{m}")
                    nc.scalar.activation(z[:], u_t[:, m, :], EXP,
                                         bias=0.0, scale=gscale[:, m:m + 1])
                    z3 = z[:].rearrange("p (r w) -> p r w", w=W)
                    wsum = wst.tile([P, NS // W], FP32, tag="ws",
                                    name=f"ws_{rep}_{m}")
                    nc.vector.tensor_reduce(wsum[:], z3, axis=AX,
                                            op=mybir.AluOpType.add)
                    wrecip = wst.tile([P, NS // W], FP32, tag="wr",
                                      name=f"wr_{rep}_{m}")
                    nc.vector.reciprocal(wrecip[:], wsum[:])
                    wb = wrecip[:].unsqueeze(2).broadcast_to((P, NS // W, W))
                    nc.gpsimd.tensor_tensor(z3, z3, wb,
                                            op=mybir.AluOpType.mult)
                    nc.sync.dma_start(o_d[m * P:(m + 1) * P, :], z[:])

    nc.compile()
    return nc


_module_cache = {}


def _get_module(repeat: int = 1, fp8: bool = True, collectives: bool = True):
    key = (repeat, fp8, collectives)
    if key not in _module_cache:
        _module_cache[key] = build_module(repeat, fp8, collectives)
    return _module_cache[key]


def make_in_maps(x: np.ndarray, fp8: bool = True):
    in_dt = ml_dtypes.float8_e4m3
    F = np.ascontiguousarray(x, dtype=np.float32).reshape(-1)
    A = F.reshape(N, C)
    Bm = F.reshape(C, N)
    in_maps = []
    for k in range(N_CORES):
        sl = slice(k * NS, (k + 1) * NS)
        b_f32 = np.ascontiguousarray(Bm[:, sl])
        b_lp = b_f32.astype(in_dt)
        bt_lp = np.ascontiguousarray(b_lp.T)
        # pre-scale A by 1/8 so per-core partial sums of S/8 fit fp8e4
        a_lp = (A[sl] * 0.125).astype(in_dt)
        in_maps.append({
            "a_in": a_lp,
            "bt_in": bt_lp,
            "b_in": b_lp,
        })
    return in_maps


def assemble_output(x: np.ndarray, results):
    term = np.concatenate(
        [results[k]["o_out"].astype(np.float32) for k in range(N_CORES)],
        axis=1)
    return (np.asarray(x, dtype=np.float32)
            + term.reshape(B, C, H, W))


def kernel(x: np.ndarray) -> np.ndarray:
    nc = _get_module()
    in_maps = make_in_maps(x)
    res = bass_utils.run_bass_kernel_spmd(
        nc, in_maps, core_ids=list(range(N_CORES)))
    return assemble_output(x, res.results)


# revision 14
# speedup vs baseline: 1.3176x; 1.3176x over previous
"""Trainium2 Bass kernel for the channel-interaction-attention module.

Reference computation (x: (4, 1024, 64, 64) fp32, F = x.ravel()):
    A  = F.view(16384, 1024)          # x.reshape(-1, C)
    Bm = F.view(1024, 16384)          # x.reshape(C, -1)
    S  = Bm @ A                       # (C, C)
    E  = softmax(S, axis=-1)
    U  = E @ Bm                       # (C, N)
    Y  = softmax(U, axis=-1)          # softmax over N = 16384
    out = x + softmax(Y.view(4,1024,64,64), axis=-1)   # softmax over W=64

Sharding: N = 16384 split into 8 column-shards of 2048 (one per core).
GEMM1 contracts over the shard -> per-core partial S/8 (host pre-scales A
by 1/8 so partials fit fp8e4).  A ReduceScatter hands core r row-block r
of S; the core softmaxes just those 128 rows (normalized), transposes
them via DMA-transpose, and an AllGather replicates E^T (fp8) to all
cores for GEMM2.  The N-softmax denominators use an AllGather of local
sums + a local reduce (cheaper than AllReduce).  GEMMs run fp8 DoubleRow.

The rep loop is software-pipelined at the source level (G2 of rep n is
emitted after G1 of rep n+1) so the in-order engine queues keep TensorE
busy across the collective latencies.
"""

import numpy as np
import ml_dtypes

import concourse.bass as bass
import concourse.bacc as bacc
import concourse.tile as tile
import concourse.mybir as mybir
from concourse import bass_utils

N_CORES = 8
B, C, H, W = 4, 1024, 64, 64
N = B * H * W            # 16384
NS = N // N_CORES        # 2048 per-core shard
P = 128
MT = C // P              # 8 row-blocks of S / U
KT1 = NS // P            # 16 contraction tiles for GEMM1
KT2 = C // P             # 8 contraction tiles for GEMM2
D1 = KT1 // 2            # 8 DoubleRow steps, GEMM1
D2 = KT2 // 2            # 4 DoubleRow steps, GEMM2

FP32 = mybir.dt.float32
BF16 = mybir.dt.bfloat16
FP8 = mybir.dt.float8e4
EXP = mybir.ActivationFunctionType.Exp
DR = mybir.MatmulPerfMode.DoubleRow
AX = mybir.AxisListType.X
RG = [list(range(N_CORES))]


def build_module(repeat: int = 1, fp8: bool = True, collectives: bool = True):
    nc = bacc.Bacc("TRN2", target_bir_lowering=False, debug=False,
                   num_devices=N_CORES if collectives else 1)

    a_d = nc.dram_tensor("a_in", [NS, C], FP8, kind="ExternalInput")
    bt_d = nc.dram_tensor("bt_in", [NS, C], FP8, kind="ExternalInput")
    b_d = nc.dram_tensor("b_in", [C, NS], FP8, kind="ExternalInput")
    o_d = nc.dram_tensor("o_out", [C, NS], BF16, kind="ExternalOutput")

    with tile.TileContext(nc) as tc:
        with (
            tc.tile_pool(name="lp", bufs=2) as lp,
            tc.tile_pool(name="upool", bufs=2) as upool,
            tc.tile_pool(name="etp", bufs=2) as etp,
            tc.tile_pool(name="ep", bufs=2) as ep,
            tc.tile_pool(name="tbp", bufs=2) as tbp,
            tc.tile_pool(name="srp", bufs=2) as srp,
            tc.tile_pool(name="scp", bufs=4) as scp,
            tc.tile_pool(name="zp", bufs=2) as zp,
            tc.tile_pool(name="wst", bufs=4) as wst,
            tc.tile_pool(name="stat", bufs=2) as stat,
            tc.tile_pool(name="cst", bufs=1) as cst,
            tc.tile_pool(name="ps1", bufs=2, space="PSUM") as psp1,
            tc.tile_pool(name="ps2", bufs=2, space="PSUM") as psp2,
            tc.tile_pool(name="dram", bufs=1, space="DRAM") as dram,
        ):
            ubias = cst.tile([P, 1], FP32, tag="ubias")
            nc.vector.memset(ubias[:], -1.5)
            st = {}

            def emit_loads(rep):
                a_t = lp.tile([P, KT1, C], FP8, tag="a")
                bt_t = lp.tile([P, KT1, C], FP8, tag="bt")
                b_t = lp.tile([P, KT2, NS], FP8, tag="b")
                nc.sync.dma_start(
                    a_t[:], a_d[:].rearrange("(k p) c -> p k c", p=P))
                nc.sync.dma_start(
                    bt_t[:], bt_d[:].rearrange("(k p) c -> p k c", p=P))
                nc.sync.dma_start(
                    b_t[:], b_d[:].rearrange("(k p) n -> p k n", p=P))
                st[rep] = {"a": a_t, "bt": bt_t, "b": b_t}

            def emit_g1_rs(rep):
                s = st[rep]
                s_in = dram.tile([MT, P, C], FP8, tag=f"si{rep}",
                                 name=f"s_in{rep}")
                rs_out = dram.tile([P, C], FP8, tag=f"sr{rep}",
                                   name=f"rs_out{rep}")
                for m in range(MT):
                    ps = psp1.tile([P, C], FP32, tag="ps1",
                                   name=f"ps1_{rep}_{m}")
                    for k in range(D1):
                        for nn in range(2):
                            nc.tensor.matmul(
                                ps[:, nn * 512:(nn + 1) * 512],
                                s["bt"][:, 2 * k:2 * k + 2,
                                        m * P:(m + 1) * P],
                                s["a"][:, 2 * k:2 * k + 2,
                                       nn * 512:(nn + 1) * 512],
                                start=(k == 0), stop=(k == D1 - 1),
                                perf_mode=DR)
                    sc = scp.tile([P, C], FP8, tag="sc",
                                  name=f"sc_{rep}_{m}")
                    nc.vector.tensor_copy(sc[:], ps[:])
                    nc.sync.dma_start(s_in[m], sc[:])
                if collectives:
                    nc.gpsimd.collective_compute(
                        "ReduceScatter", mybir.AluOpType.add,
                        replica_groups=RG,
                        ins=[s_in.opt()], outs=[rs_out.opt()])
                else:
                    nc.sync.dma_start(rs_out[:], s_in[0])
                s["rs_out"] = rs_out

            def emit_sm_ag(rep):
                s = st[rep]
                sr = srp.tile([P, C], FP8, tag="sr", name=f"sr_{rep}")
                nc.sync.dma_start(sr[:], s["rs_out"][:])
                negmax = stat.tile([P, 1], FP32, tag="nm", name=f"nm_{rep}")
                negmax8 = stat.tile([P, 1], FP32, tag="nm8",
                                    name=f"nm8_{rep}")
                rsum = stat.tile([P, 1], FP32, tag="rs", name=f"rs_{rep}")
                rscale = stat.tile([P, 1], FP32, tag="rsc",
                                   name=f"rsc_{rep}")
                nc.vector.tensor_reduce(negmax[:], sr[:], axis=AX,
                                        op=mybir.AluOpType.max, negate=True)
                nc.vector.tensor_scalar_mul(negmax8[:], negmax[:], 8.0)
                e_t = ep.tile([P, C], BF16, tag="e", name=f"e_{rep}")
                nc.scalar.activation(e_t[:], sr[:], EXP,
                                     bias=negmax8[:], scale=8.0,
                                     accum_out=rsum[:])
                nc.vector.reciprocal(rscale[:], rsum[:])
                e_n = ep.tile([P, C], BF16, tag="en", name=f"en_{rep}")
                nc.vector.tensor_scalar_mul(e_n[:], e_t[:], rscale[:])
                tb = tbp.tile([P, KT2, P], BF16, tag="tb", name=f"tb_{rep}")
                nc.sync.dma_start(tb[:], e_n[:], transpose=True)
                t8 = tbp.tile([P, KT2, P], FP8, tag="t8", name=f"t8_{rep}")
                nc.vector.tensor_copy(t8[:], tb[:])
                ag_in = dram.tile([KT2, P, P], FP8, tag=f"ai{rep}",
                                  name=f"ag_in{rep}")
                ag_out = dram.tile([MT, KT2, P, P], FP8, tag=f"ao{rep}",
                                   addr_space="Shared", name=f"ag_out{rep}")
                nc.sync.dma_start(ag_in[:].rearrange("k p c -> p k c"),
                                  t8[:])
                if collectives:
                    nc.gpsimd.collective_compute(
                        "AllGather", mybir.AluOpType.bypass,
                        replica_groups=RG,
                        ins=[ag_in.opt()], outs=[ag_out.opt()])
                else:
                    for q in range(MT):
                        nc.sync.dma_start(ag_out[q], ag_in[:])
                et_t = etp.tile([P, KT2, C], FP8, tag="et")
                for m in range(MT):
                    nc.sync.dma_start(
                        et_t[:, :, m * P:(m + 1) * P],
                        ag_out[m].rearrange("k p c -> p k c"))
                s["et"] = et_t

            def emit_g2(rep):
                s = st[rep]
                acc = stat.tile([P, MT, 2], FP32, tag="ac", name=f"ac_{rep}")
                lsum = stat.tile([P, MT], FP32, tag="ls", name=f"ls_{rep}")
                u_t = upool.tile([P, MT, NS], FP8, tag="u")
                for m in range(MT):
                    for np_ in range(2):
                        ps2 = psp2.tile([P, C], FP32, tag="ps2",
                                        name=f"ps2_{rep}_{m}_{np_}")
                        for k in range(D2):
                            for nn in range(2):
                                nc.tensor.matmul(
                                    ps2[:, nn * 512:(nn + 1) * 512],
                                    s["et"][:, 2 * k:2 * k + 2,
                                            m * P:(m + 1) * P],
                                    s["b"][:, 2 * k:2 * k + 2,
                                           np_ * C + nn * 512:
                                           np_ * C + (nn + 1) * 512],
                                    start=(k == 0), stop=(k == D2 - 1),
                                    perf_mode=DR)
                        # u = exp(U - 1.5): -1.5 keeps exp in fp8e4 range
                        # and cancels in the N-softmax normalization
                        nc.scalar.activation(
                            u_t[:, m, np_ * C:(np_ + 1) * C], ps2[:], EXP,
                            bias=ubias[:], scale=1.0,
                            accum_out=acc[:, m, np_:np_ + 1])
                nc.vector.tensor_reduce(lsum[:], acc[:], axis=AX,
                                        op=mybir.AluOpType.add)
                ls_in = dram.tile([P, MT], FP32, tag=f"li{rep}",
                                  name=f"ls_in{rep}")
                ls_out = dram.tile([N_CORES, P, MT], FP32, tag=f"lo{rep}",
                                   addr_space="Shared", name=f"ls_out{rep}")
                nc.sync.dma_start(ls_in[:], lsum[:])
                if collectives:
                    nc.gpsimd.collective_compute(
                        "AllGather", mybir.AluOpType.bypass,
                        replica_groups=RG,
                        ins=[ls_in.opt()], outs=[ls_out.opt()])
                else:
                    for q in range(N_CORES):
                        nc.sync.dma_start(ls_out[q], ls_in[:])
                gs8 = stat.tile([P, N_CORES, MT], FP32, tag="g8",
                                name=f"g8_{rep}")
                gsum = stat.tile([P, MT], FP32, tag="gs", name=f"gs_{rep}")
                gscale = stat.tile([P, MT], FP32, tag="gsc",
                                   name=f"gsc_{rep}")
                nc.sync.dma_start(gs8[:],
                                  ls_out[:].rearrange("r p m -> p r m"))
                nc.vector.tensor_reduce(gsum[:],
                                        gs8[:].rearrange("p r m -> p m r"),
                                        axis=AX, op=mybir.AluOpType.add)
                nc.vector.reciprocal(gscale[:], gsum[:])
                s["u"] = u_t
                s["gscale"] = gscale

            def emit_z(rep):
                s = st[rep]
                for m in range(MT):
                    z = zp.tile([P, NS], BF16, tag="z", name=f"z_{rep}_{m}")
                    nc.scalar.activation(z[:], s["u"][:, m, :], EXP,
                                         bias=0.0,
                                         scale=s["gscale"][:, m:m + 1])
                    z3 = z[:].rearrange("p (r w) -> p r w", w=W)
                    wsum = wst.tile([P, NS // W], FP32, tag="ws",
                                    name=f"ws_{rep}_{m}")
                    nc.vector.tensor_reduce(wsum[:], z3, axis=AX,
                                            op=mybir.AluOpType.add)
                    wrecip = wst.tile([P, NS // W], FP32, tag="wr",
                                      name=f"wr_{rep}_{m}")
                    nc.vector.reciprocal(wrecip[:], wsum[:])
                    wb = wrecip[:].unsqueeze(2).broadcast_to(
                        (P, NS // W, W))
                    nc.gpsimd.tensor_tensor(z3, z3, wb,
                                            op=mybir.AluOpType.mult)
                    nc.sync.dma_start(o_d[m * P:(m + 1) * P, :], z[:])
                del st[rep]

            # software-pipelined emission: G2/z of rep n-1 come after
            # G1/RS of rep n so in-order engine queues overlap them
            for rep in range(repeat):
                emit_loads(rep)
                emit_g1_rs(rep)
                emit_sm_ag(rep)
                if rep > 0:
                    emit_g2(rep - 1)
                    emit_z(rep - 1)
            emit_g2(repeat - 1)
            emit_z(repeat - 1)

    nc.compile()
    return nc


_module_cache = {}


def _get_module(repeat: int = 1, fp8: bool = True, collectives: bool = True):
    key = (repeat, fp8, collectives)
    if key not in _module_cache:
        _module_cache[key] = build_module(repeat, fp8, collectives)
    return _module_cache[key]


def make_in_maps(x: np.ndarray, fp8: bool = True):
    in_dt = ml_dtypes.float8_e4m3
    F = np.ascontiguousarray(x, dtype=np.float32).reshape(-1)
    A = F.reshape(N, C)
    Bm = F.reshape(C, N)
    in_maps = []
    for k in range(N_CORES):
        sl = slice(k * NS, (k + 1) * NS)
        b_f32 = np.ascontiguousarray(Bm[:, sl])
        b_lp = b_f32.astype(in_dt)
        bt_lp = np.ascontiguousarray(b_lp.T)
        # pre-scale A by 1/8 so per-core partial sums of S/8 fit fp8e4
        a_lp = (A[sl] * 0.125).astype(in_dt)
        in_maps.append({
            "a_in": a_lp,
            "bt_in": bt_lp,
            "b_in": b_lp,
        })
    return in_maps


def assemble_output(x: np.ndarray, results):
    term = np.concatenate(
        [results[k]["o_out"].astype(np.float32) for k in range(N_CORES)],
        axis=1)
    return (np.asarray(x, dtype=np.float32)
            + term.reshape(B, C, H, W))


def kernel(x: np.ndarray) -> np.ndarray:
    nc = _get_module()
    in_maps = make_in_maps(x)
    res = bass_utils.run_bass_kernel_spmd(
        nc, in_maps, core_ids=list(range(N_CORES)))
    return assemble_output(x, res.results)
